# revision 20
# baseline (speedup 1.0000x reference)
"""Bass/Tile Trainium2 kernel for nn_Attention (B=4, T=4096, C=256), 8 cores.

Sharding: core = (batch b, query-half h). Each core computes its batch's
key-side tensors and attention output for its 2048 query rows.

Key compaction: the 0/1 key mask keeps ~50% of keys. The host gathers the
valid key columns of x^T per batch (padded with zeros to TK=2176), so the
device only projects/attends over 17 key blocks instead of 32 — softmax
over the compacted key set is exact (the torch +1.0-on-valid-keys quirk is
a uniform shift that cancels; padding keys have v=0 and a zeroed
ones-column entry so they drop out of both softmax sums). Falls back to a
full-T build if a batch ever has more than TK valid keys.

Fused score weight: scoresT = x_k^T (Wk^T Wq) x_q, so the host ships the
single [C,C] matrix M^T = Wq^T Wk and the device runs ONE query-side
projection xm = M^T x_q; the score matmul's stationary operand is the raw
compacted x^T already in SBUF.

DMA: all inputs are host-packed into the exact SBUF layout
([128 partitions, contiguous free dim]) so every dma_start is a plain 2D
contiguous block with >=1KB per-partition lines, split into pieces across
the sync/scalar HWDGE queues so compute starts ~2us in and the remaining
stream hides under the first superblock. (The previous strided-descriptor
layout measured 95 GB/s and a 23.5us serial startup.)

Layout (all matmuls bf16, fp32 PSUM accumulation):
  - scoresT comes out [keys j on partitions, queries q on free dim], so exp
    needs no transposes and softmax needs no partition reductions (and no
    max-subtraction: scores are O(1) so fp32 exp cannot overflow).
  - V gets a column of ones appended: out[q, 257] accumulates the softmax
    denominator for free. Final: out[:, :256] * (1/out[:, 256]).
  - Main loop per key block jb: 2 score matmuls (FD=512) + 4 out-matmuls
    (FD=257), software-pipelined with the score stream running three
    blocks ahead so ACT's exp (~700 ns/tile) stays off the critical path.
  - The v-projection and the sb>=1 part of the q-projection are interleaved
    into superblock 0's key-block loop, so they overlap the input-DMA tail
    instead of serializing at startup; only superblock 0's 4-matmul
    q-projection runs before the main loop.
  - This kernel is PE-bound at its MAC floor: 156.4k matmul cycles at the
    platform's measured ~1.48GHz effective PE clock (8 cores active)
    accounts for ~106us; DMA, exp, LDW and normalization all hide under it.
  - Every superblock ends qb-grouped: each out-psum tile gets its final TG
    accumulations, normalization (split DVE/ACT) and store emitted per-qb
    while the PE still works on the later tiles, so the next superblock's
    matmuls never stall on psum recycling.
  - Output is stored bf16 in [sb, p, qb, c] order (one contiguous 4KB-line
    DMA per superblock); the host unpermutes and upcasts.
"""

import numpy as np
import ml_dtypes

import concourse.bacc as bacc
import concourse.mybir as mybir
import concourse.tile as tile
from concourse.bass_utils import run_bass_kernel_spmd

B, T, C = 4, 4096, 256
NCORES = 8
HALVES = NCORES // B          # 2 query-halves per batch
TQ = T // HALVES              # 2048 query rows per core
PB = 128                      # partition block
NCCH = C // PB                # 2 contraction chunks of 128
TK = 2176                     # compacted+padded key count (17 blocks of 128)
SBW = 512                     # query superblock width
NSB = TQ // SBW               # 4 superblocks per core
NQB = SBW // PB               # 4 query 128-blocks per superblock
VW = C + 1                    # v tile width incl. ones column
SCALE = float(C) ** -0.5
BF16 = mybir.dt.bfloat16
F32 = mybir.dt.float32
TQH = TQ // 2                 # xq DMA piece width

_NOLDW = []               # matmul names whose Ldweights should be stripped


def _mm(nc, out, lhsT, rhs, reuse=False, **kw):
    h = nc.tensor.matmul(out, lhsT=lhsT, rhs=rhs, **kw)
    if reuse:
        _NOLDW.append(h.ins.name)
    return h


def _strip_reused_ldweights(nc):
    """Remove InstLdweights preceding matmuls that reuse the loaded stationary.

    The PE keeps the stationary operand loaded across matmuls; a matmul whose
    lhsT is identical to the previous matmul's does not need its own weight
    load. bass always emits an Ldweights per matmul; stripping redundant
    loads saves the (mostly but not fully hidden) load time and queue slots.
    Deps of the stripped Ldweights are merged into the matmul; dangling dep
    references are remapped.
    """
    mm_names = set(_NOLDW)
    _NOLDW.clear()
    if not mm_names:
        return
    removed = {}
    for blk in nc.main_func.blocks:
        insts = blk.instructions
        i = 0
        while i < len(insts):
            inst = insts[i]
            if type(inst).__name__ == "InstMatmult" and inst.name in mm_names:
                assert i > 0 and type(insts[i - 1]).__name__ == "InstLdweights"
                ldw = insts[i - 1]
                deps = inst.sync_dependency_set_copy()
                deps.update(ldw.sync_dependency_set_copy())
                inst.set_sync_dependencies(deps)
                removed[ldw.name] = inst.name
                del insts[i - 1]
                i -= 1
            i += 1
    for blk in nc.main_func.blocks:
        for inst in blk.instructions:
            inst.remap_dependency_names(removed)


def _emit(tc, out, xt, xq, mt, wv, mb, tk, mode="full"):
    nc = tc.nc
    import contextlib
    njb = tk // PB            # key blocks

    with contextlib.ExitStack() as ctx:
        persist = ctx.enter_context(tc.tile_pool(name="persist", bufs=1))
        # Persistent SBUF tensors; c-chunks laid side by side on the free
        # dim, matching the host-packed DRAM layout exactly.
        xt_sb = persist.tile([PB, NCCH * tk], BF16)   # x^T  (compacted keys)
        xq_sb = persist.tile([PB, NCCH * TQ], BF16)   # x^T  (this core's half)
        mt_sb = persist.tile([PB, NCCH * C], BF16)    # (Wq^T Wk) fused weight
        wv_sb = persist.tile([PB, NCCH * C], BF16)
        xm_sb = persist.tile([PB, NCCH * TQ], BF16)   # M^T x_q  (query-side)
        va_sb = persist.tile([PB, njb * VW], BF16)    # masked v + ones col
        mb_sb = persist.tile([PB, njb], F32)          # 0/1 mask, [j in block, jb]

        # ---- input DMAs: plain 2D contiguous pieces, pipelined.
        # sync and scalar HWDGE queues stream in parallel; gpsimd (SWDGE)
        # takes the small tensors needed later. Piece order per queue is the
        # consumption order: mt+xq(t<1024) feed the first q-projection, the
        # leading xt blocks feed the early v-projections interleaved into
        # superblock 0, the rest streams under the main loop.
        kA = min(4 * PB, tk)              # first xt piece: 4 key blocks
        kB = min(12 * PB, tk)
        q_engs = (nc.sync, nc.scalar)
        nc.sync.dma_start(mt_sb[:], mt)
        # xq pieces: the first 512 queries feed superblock 0's projection
        # (the only one done before the main loop); the rest arrives under
        # superblock 0 and is projected by the interleaved xm units.
        qcuts = (0, SBW, TQH, TQ)
        for qi in range(2):
            for cc in range(NCCH):
                q_engs[cc].dma_start(
                    xq_sb[:, cc * TQ + qcuts[qi]: cc * TQ + qcuts[qi + 1]],
                    xq[:, cc * TQ + qcuts[qi]: cc * TQ + qcuts[qi + 1]])
            if qi == 0:
                for cc in range(NCCH):
                    q_engs[cc].dma_start(
                        xt_sb[:, cc * tk: cc * tk + kA],
                        xt[:, cc * tk: cc * tk + kA])
        for cc in range(NCCH):
            q_engs[cc].dma_start(
                xt_sb[:, cc * tk + kA: cc * tk + kB],
                xt[:, cc * tk + kA: cc * tk + kB])
        for cc in range(NCCH):
            q_engs[cc].dma_start(
                xq_sb[:, cc * TQ + TQH: (cc + 1) * TQ],
                xq[:, cc * TQ + TQH: (cc + 1) * TQ])
        if kB < tk:
            for cc in range(NCCH):
                q_engs[cc].dma_start(
                    xt_sb[:, cc * tk + kB: (cc + 1) * tk],
                    xt[:, cc * tk + kB: (cc + 1) * tk])
        nc.gpsimd.dma_start(wv_sb[:], wv)
        nc.gpsimd.dma_start(mb_sb[:], mb)
        # masked ones column on gpsimd: same queue as the mb DMA, keeps
        # DVE/ACT queues free for the projection copies.
        va_ones = va_sb[:].rearrange("p (j e) -> p j e", e=VW)[:, :, C:C + 1]
        nc.gpsimd.tensor_copy(va_ones, mb_sb[:].rearrange("p (j e) -> p j e", e=1))

        fin = ctx.enter_context(tc.tile_pool(name="fin", bufs=3))

        if mode == "dmaonly":
            os_t = fin.tile([PB, NQB * C], BF16, tag="os", name="os_t")
            nc.vector.memset(os_t[:], 0.0)
            for i, t in enumerate((xt_sb, xq_sb, mt_sb, wv_sb, mb_sb)):
                nc.vector.tensor_copy(os_t[:, i:i + 1], t[:, 0:1])
            nc.vector.tensor_copy(os_t[:, 5:6], xt_sb[:, NCCH * tk - 1:])
            nc.vector.tensor_copy(os_t[:, 6:7], xq_sb[:, NCCH * TQ - 1:])
            nc.sync.dma_start(out[0:PB, :], os_t)
            return

        # ---- q-projection for superblock 0 only (first 512 queries) ----
        # xm[c', q] = sum_c Mt[c, c'] xq[c, q]; the remaining 3 superblocks'
        # projections run as xm units interleaved into superblock 0's loop.
        with tc.tile_pool(name="projq", bufs=1, space="PSUM") as pq:
            ps_q = [pq.tile([PB, SBW], F32, tag=f"pq{dc}", name="pq_t")
                    for dc in range(NCCH)]
            for dc in range(NCCH):
                for cc in range(NCCH):
                    _mm(nc, ps_q[dc],
                        mt_sb[:, cc * C + dc * PB: cc * C + (dc + 1) * PB],
                        xq_sb[:, cc * TQ: cc * TQ + SBW],
                        start=(cc == 0), stop=(cc == NCCH - 1))
            cp_engs = (nc.vector.tensor_copy, nc.scalar.copy)
            for dc in range(NCCH):
                cp_engs[dc % 2](xm_sb[:, dc * TQ: dc * TQ + SBW], ps_q[dc])

        # ---- main attention loop ----
        # v-proj/xm-unit psum tiles share the score pool's rotating buffers
        # (tag "sc"), so the score pipeline keeps depth 4 and PSUM stays at
        # 8 banks: 4 sc + 4 out.
        scp = ctx.enter_context(tc.tile_pool(name="sc_psum", bufs=4, space="PSUM"))
        op = ctx.enter_context(tc.tile_pool(name="o_psum", bufs=1, space="PSUM"))
        # p tiles live: 3 held for the current tail + up to 4 of the next
        # superblock's pipeline
        ppool = ctx.enter_context(tc.tile_pool(name="p_pool", bufs=8))

        npair = (njb + 1) // 2

        static_p = None
        if mode in ("mmonly", "mmonly1", "nodeps"):
            # timing probes: out-matmuls read a constant p (no dependency on
            # the exp stream); mmonly additionally skips the exps entirely,
            # mmonly1 also strips the out-matmul weight loads.
            static_p = persist.tile([PB, SBW], BF16, name="static_p")
            nc.vector.memset(static_p[:], 0.01)

        def emit_vpair(jp):
            # v[t, d] for key blocks 2jp, 2jp+1: lhsT = x^T block, rhs = W^T.
            # xt is host-compacted (pad rows are 0; the ones column carries
            # the pad mask). Two blocks share one psum tile so the DVE copy
            # moves 512 columns at once.
            pair = min(2, njb - 2 * jp)
            psv = scp.tile([PB, 512], F32, tag="sc", name="psv_t")
            for j in range(pair):
                for cc in range(NCCH):
                    _mm(nc, psv[:, j * C:(j + 1) * C],
                        xt_sb[:, cc * tk + (2 * jp + j) * PB:
                              cc * tk + (2 * jp + j + 1) * PB],
                        wv_sb[:, cc * C:(cc + 1) * C],
                        start=(cc == 0), stop=(cc == NCCH - 1))
            dstv = va_sb[:, 2 * jp * VW:(2 * jp + pair) * VW].rearrange(
                "p (j e) -> p j e", e=VW)[:, :, 0:C]
            srcv = psv[:, 0:pair * C].rearrange("p (j e) -> p j e", e=C)
            nc.vector.tensor_copy(dstv, srcv)

        xm_units = [(dc, ts) for ts in range(1, TQ // SBW) for dc in range(NCCH)]

        def emit_xm_unit(u):
            # one 512-wide q slice x one output c-chunk of the remaining
            # q-projection, interleaved into superblock 0's key loop.
            dc, ts = xm_units[u]
            psx = scp.tile([PB, SBW], F32, tag="sc", name="psx_t")
            for cc in range(NCCH):
                _mm(nc, psx,
                    mt_sb[:, cc * C + dc * PB: cc * C + (dc + 1) * PB],
                    xq_sb[:, cc * TQ + ts * SBW: cc * TQ + (ts + 1) * SBW],
                    start=(cc == 0), stop=(cc == NCCH - 1))
            nc.vector.tensor_copy(
                xm_sb[:, dc * TQ + ts * SBW: dc * TQ + (ts + 1) * SBW], psx)

        # score stream indexed globally g = sb*njb + jb so the exp pipeline
        # carries ACROSS superblock boundaries: the next superblock's first
        # scores+exps are emitted during the current superblock's qb-grouped
        # tail, so its first out-matmuls never wait on a pipeline refill.
        NG = NSB * njb
        p_tiles = {}

        def emit_scores_g(g):
            sb, jb = divmod(g, njb)
            ps = scp.tile([PB, SBW], F32, tag="sc", name="sc_ps")
            for cc in range(NCCH):
                _mm(nc, ps,
                    xt_sb[:, cc * tk + jb * PB: cc * tk + (jb + 1) * PB],
                    xm_sb[:, cc * TQ + sb * SBW: cc * TQ + (sb + 1) * SBW],
                    start=(cc == 0), stop=(cc == NCCH - 1))
            if mode in ("mmonly", "mmonly1"):
                p_tiles[g] = None
                return
            pt = ppool.tile([PB, SBW], BF16, tag="p", name="p_t")
            nc.scalar.activation(
                pt, ps, mybir.ActivationFunctionType.Exp, scale=SCALE)
            p_tiles[g] = pt

        gsc = [0]

        def top_up(g_needed):
            while gsc[0] <= min(g_needed, NG - 1):
                emit_scores_g(gsc[0])
                gsc[0] += 1

        TG = min(3, njb - 1)
        nvp = [2]
        nxm = [0]
        for sb in range(NSB):
            op_tiles = [op.tile([PB, VW], F32, tag=f"o{qb}", name=f"opsum{qb}")
                        for qb in range(NQB)]

            def emit_out(jb, qbs=range(NQB), op_tiles=op_tiles, sb=sb,
                         pop=True):
                pt = p_tiles.pop(sb * njb + jb) if pop else p_tiles[sb * njb + jb]
                if static_p is not None:
                    pt = static_p
                if mode == "noout":
                    return
                for qb in qbs:
                    _mm(nc, op_tiles[qb],
                        pt[:, qb * PB:(qb + 1) * PB],
                        va_sb[:, jb * VW:(jb + 1) * VW],
                        reuse=(mode == "mmonly1" and not (jb == 0 and qb == 0)),
                        start=(jb == 0), stop=(jb == njb - 1))

            if sb == 0:
                top_up(2)
                emit_vpair(0)
                emit_vpair(1)
            for jb in range(njb - TG):
                top_up(sb * njb + jb + 3)
                if sb == 0:
                    if jb % 2 == 0 and nvp[0] < npair:
                        emit_vpair(nvp[0])
                        nvp[0] += 1
                    elif nxm[0] < len(xm_units):
                        emit_xm_unit(nxm[0])
                        nxm[0] += 1
                emit_out(jb)
            top_up(sb * njb + njb - 1)
            if sb == 0:
                while nvp[0] < npair:
                    emit_vpair(nvp[0])
                    nvp[0] += 1
                while nxm[0] < len(xm_units):
                    emit_xm_unit(nxm[0])
                    nxm[0] += 1
            # qb-grouped ending: each out-psum tile gets its final TG
            # accumulations, normalization and store slice while the PE
            # still works on the later tiles; the next superblock's score
            # stream is topped up between qb groups so its exps finish
            # before that superblock's first out-matmuls.
            os_t = fin.tile([PB, NQB * C], BF16, tag="os", name="os_t")
            for qb in range(NQB):
                for jb in range(njb - TG, njb):
                    emit_out(jb, qbs=(qb,), pop=(qb == NQB - 1))
                top_up((sb + 1) * njb + qb)
                if mode == "noout":
                    nc.vector.tensor_copy(
                        os_t[:, qb * C:(qb + 1) * C],
                        xm_sb[:, sb * SBW + qb * PB: sb * SBW + qb * PB + C])
                    continue
                rec = fin.tile([PB, 1], F32, tag="rec", name="rec_t")
                nc.vector.reciprocal(rec, op_tiles[qb][:, C:C + 1])
                osq = os_t[:, qb * C:(qb + 1) * C]
                if qb % 2 == 1:
                    nc.scalar.activation(
                        osq, op_tiles[qb][:, 0:C],
                        mybir.ActivationFunctionType.Copy, scale=rec[:])
                else:
                    nc.vector.tensor_scalar_mul(osq, op_tiles[qb][:, 0:C], rec)
            dma_eng = nc.gpsimd if sb % 2 == 0 else nc.sync
            dma_eng.dma_start(out[sb * PB:(sb + 1) * PB, :], os_t)


def build_nc(reps=1, loop_n=0, mode="full", tk=TK):
    nc = bacc.Bacc("TRN2", target_bir_lowering=False, debug=False)
    xt = nc.dram_tensor("xt", [PB, NCCH * tk], BF16, kind="ExternalInput").ap()
    xq = nc.dram_tensor("xq", [PB, NCCH * TQ], BF16, kind="ExternalInput").ap()
    mt = nc.dram_tensor("mt", [PB, NCCH * C], BF16, kind="ExternalInput").ap()
    wv = nc.dram_tensor("wv", [PB, NCCH * C], BF16, kind="ExternalInput").ap()
    mb = nc.dram_tensor("mb", [PB, tk // PB], F32, kind="ExternalInput").ap()
    out = nc.dram_tensor("out", [NSB * PB, NQB * C], BF16, kind="ExternalOutput").ap()
    with tile.TileContext(nc) as tc:
        if loop_n:
            with tc.For_i(0, loop_n, 1, hint_engines=(mybir.EngineType.PE,)):
                _emit(tc, out, xt, xq, mt, wv, mb, tk, mode=mode)
        else:
            for _ in range(reps):
                _emit(tc, out, xt, xq, mt, wv, mb, tk, mode=mode)
    _strip_reused_ldweights(nc)
    nc.compile()
    return nc


_CACHE = {}


def _get_nc(tk=TK):
    key = ("nc", tk)
    if key not in _CACHE:
        _CACHE[key] = build_nc(tk=tk)
    return _CACHE[key]


def _pack_rows(a):
    """[256, W] -> [128, 2*W]: c-chunks side by side on the free dim."""
    w = a.shape[1]
    return np.ascontiguousarray(
        a.reshape(NCCH, PB, w).transpose(1, 0, 2).reshape(PB, NCCH * w))


def make_in_maps(x, mask, tk=None):
    bf = ml_dtypes.bfloat16
    x = np.asarray(x, dtype=np.float32)
    m = np.asarray(mask) != 0                                    # [B, T]
    counts = m.sum(axis=1)
    if tk is None:
        tk = TK if counts.max() <= TK else T                     # fallback: no compaction
    xt_all = np.ascontiguousarray(x.transpose(0, 2, 1)).astype(bf)  # [B, C, T]
    maps = []
    xtc_all, mbc_all = [], []
    for b in range(B):
        idx = np.nonzero(m[b])[0]
        nv = len(idx)
        xtc = np.zeros((C, tk), dtype=bf)
        xtc[:, :nv] = xt_all[b][:, idx]
        mbc = np.zeros(tk, dtype=np.float32)
        mbc[:nv] = 1.0
        xtc_all.append(_pack_rows(xtc))
        mbc_all.append(np.ascontiguousarray(mbc.reshape(tk // PB, PB).T))
    for core in range(NCORES):
        b, h = divmod(core, HALVES)
        maps.append({
            "xt": xtc_all[b],
            "xq": _pack_rows(xt_all[b][:, h * TQ:(h + 1) * TQ]),
            "mb": mbc_all[b],
        })
    return maps, tk


def make_wt_maps(Wk, Wq, Wv):
    bf = ml_dtypes.bfloat16
    wq32 = np.asarray(Wq, dtype=np.float32)
    wk32 = np.asarray(Wk, dtype=np.float32)
    # scoresT[k, q] = sum_c xt[c,k] xm[c,q], xm = Mt^T xq, Mt[c',c] = (Wq^T Wk)[c',c]
    mt = np.ascontiguousarray(wq32.T @ wk32).astype(bf)
    wvt = np.ascontiguousarray(np.asarray(Wv, dtype=np.float32).T).astype(bf)
    return {"mt": _pack_rows(mt), "wv": _pack_rows(wvt)}


def kernel(x, mask, Wk, Wq, Wv):
    in_maps, tk = make_in_maps(x, mask)
    wts = make_wt_maps(Wk, Wq, Wv)
    for m in in_maps:
        m.update(wts)
    res = run_bass_kernel_spmd(_get_nc(tk), in_maps, list(range(NCORES)))
    out = np.empty((B, T, C), np.float32)
    for core in range(NCORES):
        b, h = divmod(core, HALVES)
        o = np.asarray(res.results[core]["out"])  # [NSB*PB, NQB*C] bf16
        o = o.reshape(NSB, PB, NQB, C).transpose(0, 2, 1, 3).reshape(TQ, C)
        out[b, h * TQ:(h + 1) * TQ, :] = o.astype(np.float32)
    return out


# revision 26
# speedup vs baseline: 1.1527x; 1.1527x over previous
"""Bass/Tile Trainium2 kernel for nn_Attention (B=4, T=4096, C=256), 8 cores.

Sharding: core = (batch b, query-half h). Each core computes its batch's
key-side tensors and attention output for its 2048 query rows.

Key compaction: the 0/1 key mask keeps ~50% of keys. The host gathers the
valid key columns of x^T per batch (padded with zeros to TK=2176), so the
device only projects/attends over 17 key blocks instead of 32 — softmax
over the compacted key set is exact (the torch +1.0-on-valid-keys quirk is
a uniform shift that cancels; padding keys have v=0 and a zeroed
ones-column entry so they drop out of both softmax sums). Falls back to a
full-T build if a batch ever has more than TK valid keys.

Fused score weight: scoresT = x_k^T (Wk^T Wq) x_q, so the host ships the
single [C,C] matrix M^T = Wq^T Wk and the device runs ONE query-side
projection xm = M^T x_q; the score matmul's stationary operand is the raw
compacted x^T already in SBUF.

DMA: all inputs are host-packed into the exact SBUF layout
([128 partitions, contiguous free dim]) so every dma_start is a plain 2D
contiguous block with >=1KB per-partition lines, split into pieces across
the sync/scalar HWDGE queues so compute starts ~2us in and the remaining
stream hides under the first superblock. (The previous strided-descriptor
layout measured 95 GB/s and a 23.5us serial startup.)

Layout (all matmuls bf16, fp32 PSUM accumulation):
  - scoresT comes out [keys j on partitions, queries q on free dim], so exp
    needs no transposes and softmax needs no partition reductions (and no
    max-subtraction: scores are O(1) so fp32 exp cannot overflow).
  - V gets a column of ones appended: out[q, 257] accumulates the softmax
    denominator for free. Final: out[:, :256] * (1/out[:, 256]).
  - Main loop per key block jb: 2 score matmuls (FD=512) + 4 out-matmuls
    (FD=257), software-pipelined with the score stream running three
    blocks ahead so ACT's exp (~700 ns/tile) stays off the critical path.
  - The v-projection and the sb>=1 part of the q-projection are interleaved
    into superblock 0's key-block loop, so they overlap the input-DMA tail
    instead of serializing at startup; only superblock 0's 4-matmul
    q-projection runs before the main loop.
  - This kernel is PE-bound at its MAC floor: 156.4k matmul cycles at the
    platform's measured ~1.48GHz effective PE clock (8 cores active)
    accounts for ~106us; DMA, exp, LDW and normalization all hide under it.
  - Every superblock ends qb-grouped: each out-psum tile gets its final TG
    accumulations, normalization (split DVE/ACT) and store emitted per-qb
    while the PE still works on the later tiles, so the next superblock's
    matmuls never stall on psum recycling.
  - Output is stored bf16 in [sb, p, qb, c] order (one contiguous 4KB-line
    DMA per superblock); the host unpermutes and upcasts.
"""

import numpy as np
import ml_dtypes

import concourse.bacc as bacc
import concourse.mybir as mybir
import concourse.tile as tile
from concourse.bass_utils import run_bass_kernel_spmd

B, T, C = 4, 4096, 256
NCORES = 8
HALVES = NCORES // B          # 2 query-halves per batch
TQ = T // HALVES              # 2048 query rows per core
PB = 128                      # partition block
NCCH = C // PB                # 2 contraction chunks of 128
TK = 2176                     # compacted+padded key count (17 blocks of 128)
SBW = 512                     # query superblock width
NSB = TQ // SBW               # 4 superblocks per core
NQB = SBW // PB               # 4 query 128-blocks per superblock
VW = C + 1                    # v tile width incl. ones column
SCALE = float(C) ** -0.5
BF16 = mybir.dt.bfloat16
F32 = mybir.dt.float32
TQH = TQ // 2                 # xq DMA piece width

_NOLDW = []               # matmul names whose Ldweights should be stripped


def _mm(nc, out, lhsT, rhs, reuse=False, **kw):
    h = nc.tensor.matmul(out, lhsT=lhsT, rhs=rhs, **kw)
    if reuse:
        _NOLDW.append(h.ins.name)
    return h


def _strip_reused_ldweights(nc):
    """Remove InstLdweights preceding matmuls that reuse the loaded stationary.

    The PE keeps the stationary operand loaded across matmuls; a matmul whose
    lhsT is identical to the previous matmul's does not need its own weight
    load. bass always emits an Ldweights per matmul; stripping redundant
    loads saves the (mostly but not fully hidden) load time and queue slots.
    Deps of the stripped Ldweights are merged into the matmul; dangling dep
    references are remapped.
    """
    mm_names = set(_NOLDW)
    _NOLDW.clear()
    if not mm_names:
        return
    removed = {}
    for blk in nc.main_func.blocks:
        insts = blk.instructions
        i = 0
        while i < len(insts):
            inst = insts[i]
            if type(inst).__name__ == "InstMatmult" and inst.name in mm_names:
                assert i > 0 and type(insts[i - 1]).__name__ == "InstLdweights"
                ldw = insts[i - 1]
                deps = inst.sync_dependency_set_copy()
                deps.update(ldw.sync_dependency_set_copy())
                inst.set_sync_dependencies(deps)
                removed[ldw.name] = inst.name
                del insts[i - 1]
                i -= 1
            i += 1
    for blk in nc.main_func.blocks:
        for inst in blk.instructions:
            inst.remap_dependency_names(removed)


def _emit(tc, out, xt, xq, mt, wv, mb, tk, mode="full", xsb=False, io=None):
    nc = tc.nc
    import contextlib
    njb = tk // PB            # key blocks

    with contextlib.ExitStack() as ctx:
        persist = ctx.enter_context(tc.tile_pool(name="persist", bufs=1))
        # Persistent SBUF tensors; c-chunks laid side by side on the free
        # dim, matching the host-packed DRAM layout exactly.
        # In the looped (benchmark) build, xt/xq live in an outer ping-pong
        # pool and the constant mt/wv/mb are resident (io != None): the next
        # rep's big input DMAs then prefetch under the current rep's compute
        # instead of serializing at the rep boundary.
        if io is None:
            xt_sb = persist.tile([PB, NCCH * tk], BF16)   # x^T (compacted keys)
            xq_sb = persist.tile([PB, NCCH * TQ], BF16)   # x^T (core's half)
            mt_sb = persist.tile([PB, NCCH * C], BF16)    # (Wq^T Wk) fused
            wv_sb = persist.tile([PB, NCCH * C], BF16)
            mb_sb = persist.tile([PB, njb], F32)          # 0/1 mask
        else:
            xt_sb, xq_sb, mt_sb, wv_sb, mb_sb = io
        xm_sb = persist.tile([PB, NCCH * TQ], BF16)   # M^T x_q  (query-side)
        va_sb = persist.tile([PB, njb * VW], BF16)    # masked v + ones col

        # ---- input DMAs: plain 2D contiguous pieces, pipelined.
        # sync and scalar HWDGE queues stream in parallel; gpsimd (SWDGE)
        # takes the small tensors needed later. Piece order per queue is the
        # consumption order: mt+xq(t<1024) feed the first q-projection, the
        # leading xt blocks feed the early v-projections interleaved into
        # superblock 0, the rest streams under the main loop.
        kA = min(4 * PB, tk)              # first xt piece: 4 key blocks
        kB = min(12 * PB, tk)
        q_engs = (nc.sync, nc.scalar)
        if io is None:
            nc.sync.dma_start(mt_sb[:], mt)
        # xq pieces: the first 512 queries feed superblock 0's projection
        # (the only one done before the main loop); the rest arrives under
        # superblock 0 and is projected by the interleaved xm units.
        qcuts = (0, SBW, TQH, TQ)
        for qi in range(2):
            for cc in range(NCCH):
                q_engs[cc].dma_start(
                    xq_sb[:, cc * TQ + qcuts[qi]: cc * TQ + qcuts[qi + 1]],
                    xq[:, cc * TQ + qcuts[qi]: cc * TQ + qcuts[qi + 1]])
            if qi == 0:
                for cc in range(NCCH):
                    q_engs[cc].dma_start(
                        xt_sb[:, cc * tk: cc * tk + kA],
                        xt[:, cc * tk: cc * tk + kA])
        for cc in range(NCCH):
            q_engs[cc].dma_start(
                xt_sb[:, cc * tk + kA: cc * tk + kB],
                xt[:, cc * tk + kA: cc * tk + kB])
        for cc in range(NCCH):
            q_engs[cc].dma_start(
                xq_sb[:, cc * TQ + TQH: (cc + 1) * TQ],
                xq[:, cc * TQ + TQH: (cc + 1) * TQ])
        if kB < tk:
            for cc in range(NCCH):
                q_engs[cc].dma_start(
                    xt_sb[:, cc * tk + kB: (cc + 1) * tk],
                    xt[:, cc * tk + kB: (cc + 1) * tk])
        if io is None:
            nc.gpsimd.dma_start(wv_sb[:], wv)
            nc.gpsimd.dma_start(mb_sb[:], mb)
        # masked ones column on gpsimd: same queue as the mb DMA, keeps
        # DVE/ACT queues free for the projection copies.
        va_ones = va_sb[:].rearrange("p (j e) -> p j e", e=VW)[:, :, C:C + 1]
        nc.gpsimd.tensor_copy(va_ones, mb_sb[:].rearrange("p (j e) -> p j e", e=1))

        fin = ctx.enter_context(tc.tile_pool(name="fin", bufs=3))

        if mode == "dmaonly":
            os_t = fin.tile([PB, NQB * C], BF16, tag="os", name="os_t")
            nc.vector.memset(os_t[:], 0.0)
            for i, t in enumerate((xt_sb, xq_sb, mt_sb, wv_sb, mb_sb)):
                nc.vector.tensor_copy(os_t[:, i:i + 1], t[:, 0:1])
            nc.vector.tensor_copy(os_t[:, 5:6], xt_sb[:, NCCH * tk - 1:])
            nc.vector.tensor_copy(os_t[:, 6:7], xq_sb[:, NCCH * TQ - 1:])
            nc.sync.dma_start(out[0:PB, :], os_t)
            return

        # ---- q-projection for superblock 0 only (first 512 queries) ----
        # xm[c', q] = sum_c Mt[c, c'] xq[c, q]; the remaining 3 superblocks'
        # projections run as xm units interleaved into superblock 0's loop.
        with tc.tile_pool(name="projq", bufs=1, space="PSUM") as pq:
            ps_q = [pq.tile([PB, SBW], F32, tag=f"pq{dc}", name="pq_t")
                    for dc in range(NCCH)]
            for dc in range(NCCH):
                for cc in range(NCCH):
                    _mm(nc, ps_q[dc],
                        mt_sb[:, cc * C + dc * PB: cc * C + (dc + 1) * PB],
                        xq_sb[:, cc * TQ: cc * TQ + SBW],
                        start=(cc == 0), stop=(cc == NCCH - 1))
            cp_engs = (nc.vector.tensor_copy, nc.scalar.copy)
            for dc in range(NCCH):
                cp_engs[dc % 2](xm_sb[:, dc * TQ: dc * TQ + SBW], ps_q[dc])

        # ---- main attention loop ----
        # v-proj/xm-unit psum tiles share the score pool's rotating buffers
        # (tag "sc"), so the score pipeline keeps depth 4 and PSUM stays at
        # 8 banks: 4 sc + 4 out.
        scp = ctx.enter_context(tc.tile_pool(name="sc_psum", bufs=4, space="PSUM"))
        op = ctx.enter_context(tc.tile_pool(name="o_psum", bufs=1, space="PSUM"))
        # p tiles live: 3 held for the current tail + up to 4 of the next
        # superblock's pipeline
        ppool = ctx.enter_context(tc.tile_pool(name="p_pool", bufs=8))

        npair = (njb + 1) // 2

        static_p = None
        if mode in ("mmonly", "mmonly1", "nodeps"):
            # timing probes: out-matmuls read a constant p (no dependency on
            # the exp stream); mmonly additionally skips the exps entirely,
            # mmonly1 also strips the out-matmul weight loads.
            static_p = persist.tile([PB, SBW], BF16, name="static_p")
            nc.vector.memset(static_p[:], 0.01)

        def emit_vpair(jp):
            # v[t, d] for key blocks 2jp, 2jp+1: lhsT = x^T block, rhs = W^T.
            # xt is host-compacted (pad rows are 0; the ones column carries
            # the pad mask). Two blocks share one psum tile so the DVE copy
            # moves 512 columns at once.
            pair = min(2, njb - 2 * jp)
            psv = scp.tile([PB, 512], F32, tag="sc", name="psv_t")
            for j in range(pair):
                for cc in range(NCCH):
                    _mm(nc, psv[:, j * C:(j + 1) * C],
                        xt_sb[:, cc * tk + (2 * jp + j) * PB:
                              cc * tk + (2 * jp + j + 1) * PB],
                        wv_sb[:, cc * C:(cc + 1) * C],
                        start=(cc == 0), stop=(cc == NCCH - 1))
            dstv = va_sb[:, 2 * jp * VW:(2 * jp + pair) * VW].rearrange(
                "p (j e) -> p j e", e=VW)[:, :, 0:C]
            srcv = psv[:, 0:pair * C].rearrange("p (j e) -> p j e", e=C)
            nc.vector.tensor_copy(dstv, srcv)

        xm_units = [(dc, ts) for ts in range(1, TQ // SBW) for dc in range(NCCH)]

        def emit_xm_unit(u):
            # one 512-wide q slice x one output c-chunk of the remaining
            # q-projection, interleaved into superblock 0's key loop.
            dc, ts = xm_units[u]
            psx = scp.tile([PB, SBW], F32, tag="sc", name="psx_t")
            for cc in range(NCCH):
                _mm(nc, psx,
                    mt_sb[:, cc * C + dc * PB: cc * C + (dc + 1) * PB],
                    xq_sb[:, cc * TQ + ts * SBW: cc * TQ + (ts + 1) * SBW],
                    start=(cc == 0), stop=(cc == NCCH - 1))
            nc.vector.tensor_copy(
                xm_sb[:, dc * TQ + ts * SBW: dc * TQ + (ts + 1) * SBW], psx)

        # score stream indexed globally g = sb*njb + jb so the exp pipeline
        # carries ACROSS superblock boundaries: the next superblock's first
        # scores+exps are emitted during the current superblock's qb-grouped
        # tail, so its first out-matmuls never wait on a pipeline refill.
        NG = NSB * njb
        p_tiles = {}

        def emit_scores_g(g):
            sb, jb = divmod(g, njb)
            ps = scp.tile([PB, SBW], F32, tag="sc", name="sc_ps")
            for cc in range(NCCH):
                _mm(nc, ps,
                    xt_sb[:, cc * tk + jb * PB: cc * tk + (jb + 1) * PB],
                    xm_sb[:, cc * TQ + sb * SBW: cc * TQ + (sb + 1) * SBW],
                    start=(cc == 0), stop=(cc == NCCH - 1))
            if mode in ("mmonly", "mmonly1"):
                p_tiles[g] = None
                return
            pt = ppool.tile([PB, SBW], BF16, tag="p", name="p_t")
            nc.scalar.activation(
                pt, ps, mybir.ActivationFunctionType.Exp, scale=SCALE)
            p_tiles[g] = pt

        gsc = [0]

        def top_up(g_needed):
            while gsc[0] <= min(g_needed, NG - 1):
                emit_scores_g(gsc[0])
                gsc[0] += 1

        TG = min(3, njb - 1)
        nvp = [2]
        nxm = [0]
        for sb in range(NSB):
            op_tiles = [op.tile([PB, VW], F32, tag=f"o{qb}", name=f"opsum{qb}")
                        for qb in range(NQB)]

            def emit_out(jb, qbs=range(NQB), op_tiles=op_tiles, sb=sb,
                         pop=True):
                pt = p_tiles.pop(sb * njb + jb) if pop else p_tiles[sb * njb + jb]
                if static_p is not None:
                    pt = static_p
                if mode == "noout":
                    return
                for qb in qbs:
                    _mm(nc, op_tiles[qb],
                        pt[:, qb * PB:(qb + 1) * PB],
                        va_sb[:, jb * VW:(jb + 1) * VW],
                        reuse=(mode == "mmonly1" and not (jb == 0 and qb == 0)),
                        start=(jb == 0), stop=(jb == njb - 1))

            if sb == 0:
                top_up(2)
                emit_vpair(0)
                emit_vpair(1)
            for jb in range(njb - TG):
                top_up(sb * njb + jb + 3)
                if sb == 0:
                    if jb % 2 == 0 and nvp[0] < npair:
                        emit_vpair(nvp[0])
                        nvp[0] += 1
                    elif nxm[0] < len(xm_units):
                        emit_xm_unit(nxm[0])
                        nxm[0] += 1
                emit_out(jb)
            top_up(sb * njb + njb - 1)
            if sb == 0:
                while nvp[0] < npair:
                    emit_vpair(nvp[0])
                    nvp[0] += 1
                while nxm[0] < len(xm_units):
                    emit_xm_unit(nxm[0])
                    nxm[0] += 1
            # qb-grouped ending: each out-psum tile gets its final TG
            # accumulations, normalization and store slice while the PE
            # still works on the later tiles; the next superblock's score
            # stream is topped up between qb groups so its exps finish
            # before that superblock's first out-matmuls.
            os_t = fin.tile([PB, NQB * C], BF16, tag="os", name="os_t")
            for qb in range(NQB):
                for jb in range(njb - TG, njb):
                    emit_out(jb, qbs=(qb,), pop=(qb == NQB - 1))
                if xsb:
                    # measured 2-6us SLOWER on HW (A/B, all rounds): next-sb
                    # scores in the tail delay the final accumulations.
                    top_up((sb + 1) * njb + qb)
                if mode == "noout":
                    nc.vector.tensor_copy(
                        os_t[:, qb * C:(qb + 1) * C],
                        xm_sb[:, sb * SBW + qb * PB: sb * SBW + qb * PB + C])
                    continue
                rec = fin.tile([PB, 1], F32, tag="rec", name="rec_t")
                nc.vector.reciprocal(rec, op_tiles[qb][:, C:C + 1])
                osq = os_t[:, qb * C:(qb + 1) * C]
                if qb % 2 == 1:
                    nc.scalar.activation(
                        osq, op_tiles[qb][:, 0:C],
                        mybir.ActivationFunctionType.Copy, scale=rec[:])
                else:
                    nc.vector.tensor_scalar_mul(osq, op_tiles[qb][:, 0:C], rec)
            dma_eng = nc.gpsimd if sb % 2 == 0 else nc.sync
            dma_eng.dma_start(out[sb * PB:(sb + 1) * PB, :], os_t)


def build_nc(reps=1, loop_n=0, mode="full", tk=TK, xsb=False, pingpong=True):
    nc = bacc.Bacc("TRN2", target_bir_lowering=False, debug=False)
    xt = nc.dram_tensor("xt", [PB, NCCH * tk], BF16, kind="ExternalInput").ap()
    xq = nc.dram_tensor("xq", [PB, NCCH * TQ], BF16, kind="ExternalInput").ap()
    mt = nc.dram_tensor("mt", [PB, NCCH * C], BF16, kind="ExternalInput").ap()
    wv = nc.dram_tensor("wv", [PB, NCCH * C], BF16, kind="ExternalInput").ap()
    mb = nc.dram_tensor("mb", [PB, tk // PB], F32, kind="ExternalInput").ap()
    out = nc.dram_tensor("out", [NSB * PB, NQB * C], BF16, kind="ExternalOutput").ap()
    with tile.TileContext(nc) as tc:
        if loop_n and not pingpong:
            with tc.For_i(0, loop_n, 1, hint_engines=(mybir.EngineType.PE,)):
                _emit(tc, out, xt, xq, mt, wv, mb, tk, mode=mode, xsb=xsb)
        elif loop_n:
            # Steady-state loop build: 2 reps per iteration with ping-pong
            # xt/xq buffers in an outer pool, so each rep's big input DMAs
            # prefetch a full rep ahead; the constant mt/wv/mb are loaded
            # once before the loop and stay resident.
            assert loop_n % 2 == 0
            njb = tk // PB
            with tc.tile_pool(name="io2", bufs=1) as iop:
                mt_r = iop.tile([PB, NCCH * C], BF16, name="mt_r")
                wv_r = iop.tile([PB, NCCH * C], BF16, name="wv_r")
                mb_r = iop.tile([PB, njb], F32, name="mb_r")
                nc.sync.dma_start(mt_r[:], mt)
                nc.gpsimd.dma_start(wv_r[:], wv)
                nc.gpsimd.dma_start(mb_r[:], mb)
                ios = []
                for s in ("A", "B"):
                    xt_s = iop.tile([PB, NCCH * tk], BF16, name=f"xt_{s}")
                    xq_s = iop.tile([PB, NCCH * TQ], BF16, name=f"xq_{s}")
                    ios.append((xt_s, xq_s, mt_r, wv_r, mb_r))
                with tc.For_i(0, loop_n // 2, 1,
                              hint_engines=(mybir.EngineType.PE,)):
                    for s in range(2):
                        _emit(tc, out, xt, xq, mt, wv, mb, tk, mode=mode,
                              xsb=xsb, io=ios[s])
        else:
            for _ in range(reps):
                _emit(tc, out, xt, xq, mt, wv, mb, tk, mode=mode, xsb=xsb)
    _strip_reused_ldweights(nc)
    nc.compile()
    return nc


_CACHE = {}


def _get_nc(tk=TK):
    key = ("nc", tk)
    if key not in _CACHE:
        _CACHE[key] = build_nc(tk=tk)
    return _CACHE[key]


def _pack_rows(a):
    """[256, W] -> [128, 2*W]: c-chunks side by side on the free dim."""
    w = a.shape[1]
    return np.ascontiguousarray(
        a.reshape(NCCH, PB, w).transpose(1, 0, 2).reshape(PB, NCCH * w))


def make_in_maps(x, mask, tk=None):
    bf = ml_dtypes.bfloat16
    x = np.asarray(x, dtype=np.float32)
    m = np.asarray(mask) != 0                                    # [B, T]
    counts = m.sum(axis=1)
    if tk is None:
        tk = TK if counts.max() <= TK else T                     # fallback: no compaction
    xt_all = np.ascontiguousarray(x.transpose(0, 2, 1)).astype(bf)  # [B, C, T]
    maps = []
    xtc_all, mbc_all = [], []
    for b in range(B):
        idx = np.nonzero(m[b])[0]
        nv = len(idx)
        xtc = np.zeros((C, tk), dtype=bf)
        xtc[:, :nv] = xt_all[b][:, idx]
        mbc = np.zeros(tk, dtype=np.float32)
        mbc[:nv] = 1.0
        xtc_all.append(_pack_rows(xtc))
        mbc_all.append(np.ascontiguousarray(mbc.reshape(tk // PB, PB).T))
    for core in range(NCORES):
        b, h = divmod(core, HALVES)
        maps.append({
            "xt": xtc_all[b],
            "xq": _pack_rows(xt_all[b][:, h * TQ:(h + 1) * TQ]),
            "mb": mbc_all[b],
        })
    return maps, tk


def make_wt_maps(Wk, Wq, Wv):
    bf = ml_dtypes.bfloat16
    wq32 = np.asarray(Wq, dtype=np.float32)
    wk32 = np.asarray(Wk, dtype=np.float32)
    # scoresT[k, q] = sum_c xt[c,k] xm[c,q], xm = Mt^T xq, Mt[c',c] = (Wq^T Wk)[c',c]
    mt = np.ascontiguousarray(wq32.T @ wk32).astype(bf)
    wvt = np.ascontiguousarray(np.asarray(Wv, dtype=np.float32).T).astype(bf)
    return {"mt": _pack_rows(mt), "wv": _pack_rows(wvt)}


def kernel(x, mask, Wk, Wq, Wv):
    in_maps, tk = make_in_maps(x, mask)
    wts = make_wt_maps(Wk, Wq, Wv)
    for m in in_maps:
        m.update(wts)
    res = run_bass_kernel_spmd(_get_nc(tk), in_maps, list(range(NCORES)))
    out = np.empty((B, T, C), np.float32)
    for core in range(NCORES):
        b, h = divmod(core, HALVES)
        o = np.asarray(res.results[core]["out"])  # [NSB*PB, NQB*C] bf16
        o = o.reshape(NSB, PB, NQB, C).transpose(0, 2, 1, 3).reshape(TQ, C)
        out[b, h * TQ:(h + 1) * TQ, :] = o.astype(np.float32)
    return out
